# revision 5
# baseline (speedup 1.0000x reference)
"""Trainium2 Bass kernel for nn_Attention_21122649161959 (v6).

RETRO-style causal self-attention block (RMSNorm -> q/kv proj -> RoPE ->
null-kv prepend -> causal masked softmax -> out proj) for
x [2, 2048, 1024], 16 heads of 64.

Sharding: 8 NeuronCores = (batch 2) x (4 head-groups of 4 heads).
Each core computes, for its batch b and heads [h0, h0+4):
    y_partial^T = Wout[h-slice]^T @ attn_out^T          [1024, 2048]
The host sums the 4 (fp16) partials per batch, transposes, adds bias.

Design notes (v6; TimelineSim ~169 us/core vs 202 us for the v4
baseline; the wall is the PE sequencer, ~670 matmul issues x ~210 ns):
  - all-fp16 datapath (x, weights, q/k/v, rotary tables, outputs).
    fp16's 10-bit mantissa keeps the datapath error ~4e-4; bf16 was 8x
    worse at identical speed.  Weights cannot be f32r: the PE rejects
    mixed 32/16-bit operand pairs.
  - x is passed twice from the host: row-major for the RMSNorm pass
    (ACT Square+accum -> per-partition Newton rsqrt on Pool) and
    pre-transposed (xT) so projections read it directly -- no on-chip
    PE transposes.  The norm reciprocals fold into per-chunk scaled
    rotary tables (cos_eff/sin_eff) for q/k and into the V-tile copy
    (ACT activation scale) for V, so normalization costs no extra
    full-size elementwise pass.
  - de-shifted V tiles: key tile j covers real keys j*128..j*128+127;
    the causal diagonal lands exactly on tile edges (no sliver pass, no
    V boundary recompute).  The null kv is a rank-1 PSUM-opening update
    from a hoistable [1,512] score row + exp.
  - out projection packs head pairs into the full 128-partition
    contraction; softmax denominators ride along as a ones-column in
    the V tiles.
  - softmax exp is split across engines: ACT table exp for hi=0 heads,
    Schraudolph fast-exp (u16 = s*1024/ln2 + (15360-59), bitcast fp16)
    on DVE for hi=1 heads.  The sawtooth (~1.8% rms, zero-bias C) is
    the dominant error term (total rel err ~9e-3 vs the 2e-2 gate).
    fp8/DoubleRow variants were tested and rejected: fp8 weight
    quantization is a systematic W perturbation that does not average
    out over keys (measured 1.9e-2).
  - braided schedule (v4 lineage): projection/norm/output generators
    yield between PE groups and are advanced between attention key
    tiles; AV matmuls trail their scores by 2 tiles so exp latency
    never blocks the in-order PE queue; PSUM evacuations (u tiles) are
    copied out fast and normalized lazily in the next block's filler.
  - DMA: wq first, then per-k-block xT slices, cos/sin, the rest; the
    output stores are fp16 to halve the tail; outproj(3) borrows the
    idle triple-buffered score PSUM pool.
"""

import sys

sys.path.insert(0, "/opt/trn_rl_repo")

from contextlib import ExitStack

import numpy as np
import ml_dtypes

import concourse.bass as bass
import concourse.tile as tile
from concourse import bacc, mybir

F32 = mybir.dt.float32
F32R = mybir.dt.float32r
FP8 = mybir.dt.float8e4
DR = mybir.MatmulPerfMode.DoubleRow
BF16 = mybir.dt.float16
U16 = mybir.dt.uint16
AF = mybir.ActivationFunctionType
OP = mybir.AluOpType

B, N, D = 2, 2048, 1024
H, DH = 16, 64
HPC = 4
CPH = HPC * DH
NCORES = 8
NJT = 16
NCI = 4
NEG = -1e9
EPS = 1e-8
QLO = (0, 128, 256, 256)

# Schraudolph fast-exp in bf16 bit space: u16 = s * AFE + (BFE + AFE*mb)
AFE = 128.0 / np.log(2.0)
CFE = 4.1  # sawtooth centering constant (tuned numerically)
BFE = 127.0 * 128.0 - CFE

TRACE = False
TRACE_KW = {}


def build_program():
    nc = bacc.Bacc(trn_type="TRN2", num_devices=NCORES)

    x_h = nc.dram_tensor("x", [N, D], BF16, kind="ExternalInput")
    xT_h = nc.dram_tensor("xT", [D, N], BF16, kind="ExternalInput")
    wq_h = nc.dram_tensor("wq", [D, CPH], BF16, kind="ExternalInput")
    wk_h = nc.dram_tensor("wk", [D, CPH], BF16, kind="ExternalInput")
    wv_h = nc.dram_tensor("wv", [D, CPH], BF16, kind="ExternalInput")
    wo_h = nc.dram_tensor("wo", [128, 2 * D], BF16, kind="ExternalInput")
    cos_h = nc.dram_tensor("cos2", [128, N], BF16, kind="ExternalInput")
    sin_h = nc.dram_tensor("sin2", [128, N], BF16, kind="ExternalInput")
    nk_h = nc.dram_tensor("nk", [128, 2 * 33], BF16, kind="ExternalInput")
    nv_h = nc.dram_tensor("nv", [2, 128], BF16, kind="ExternalInput")
    mb_h = nc.dram_tensor("mb", [N], F32, kind="ExternalInput")
    bmb_h = nc.dram_tensor("bmb", [N], F32, kind="ExternalInput")
    yt_h = nc.dram_tensor("yt", [D, N], BF16, kind="ExternalOutput")
    rscr_h = nc.dram_tensor("rscr", [N], F32, kind="Internal")

    with ExitStack() as ctx:
        tc = ctx.enter_context(tile.TileContext(nc))
        persist = ctx.enter_context(tc.tile_pool(name="persist", bufs=1))

        def single(shape, tag, dt=F32):
            return persist.tile(shape, dt, tag=tag, name=tag)

        qt = [single([128, N], f"qt{m}", dt=BF16) for m in range(2)]
        kt = [single([128, N], f"kt{m}", dt=BF16) for m in range(2)]
        vsb = [single([128, HPC, DH + 1], f"v{j}", dt=BF16) for j in range(NJT)]
        mb_sb = single([128, NJT], "mb")
        bmb_sb = single([128, NJT], "bmb")
        cos_sb = single([128, N], "cos", dt=BF16)
        sin_sb = single([128, N], "sin", dt=BF16)
        cos_eff = single([128, N], "cose", dt=BF16)
        sin_eff = single([128, N], "sine", dt=BF16)
        rq = single([128, NJT], "rq")
        rrow = single([1, N], "rrow")
        sqd = single([128, D], "sqd", dt=BF16)

        wq_sb = single([128, 8, CPH], "wqs", dt=BF16)
        wk_sb = single([128, 8, CPH], "wks", dt=BF16)
        wv_sb = single([128, 8, CPH], "wvs", dt=BF16)
        wo_sb = single([128, 2, D], "wos", dt=BF16)
        nk_sb = single([128, 2, 33], "nks", dt=BF16)
        nv_sb = single([33, 2, DH + 1], "nvs", dt=BF16)

        # dummy exp first: pulls the ACT exp-table load off the critical path
        # without waiting on Pool's mask construction
        dexp = single([1, 1], "dexp")
        nc.vector.memset(dexp, 0.0)
        nc.scalar.activation(out=sqd[0:1, 0:1], in_=dexp, func=AF.Exp)

        masks = []
        for off in range(4):
            mt = single([128, 2, 512], f"mask{off}", dt=BF16)
            nc.gpsimd.memset(mt, 1.0)
            for hi in range(2):
                nc.gpsimd.affine_select(
                    out=mt[:, hi, :], in_=mt[:, hi, :], pattern=[[1, 512]],
                    compare_op=OP.is_ge, fill=0.0,
                    base=-off * 128, channel_multiplier=-1,
                )
            masks.append(mt)

        nc.vector.memset(nv_sb[:, :, DH:DH + 1], 1.0)
        for j in range(NJT):
            nc.vector.memset(vsb[j][:, :, DH:DH + 1], 1.0)

        with tc.tile_pool(name="xin", bufs=6) as xin, \
             tc.tile_pool(name="xnt", bufs=2) as xnt, \
             tc.tile_pool(name="stat", bufs=2) as stat, \
             tc.tile_pool(name="rope", bufs=3) as rope, \
             tc.tile_pool(name="epool", bufs=6) as epool, \
             tc.tile_pool(name="enp", bufs=2) as enp, \
             tc.tile_pool(name="npool", bufs=4) as npool, \
             tc.tile_pool(name="upool", bufs=4) as upool, \
             tc.tile_pool(name="yout", bufs=4) as yout, \
             tc.tile_pool(name="pp", bufs=2, space="PSUM") as pp, \
             tc.tile_pool(name="pss", bufs=3, space="PSUM") as pss, \
             tc.tile_pool(name="psu", bufs=2, space="PSUM") as psu, \
             tc.tile_pool(name="ypp", bufs=1, space="PSUM") as ypp:

            xcs = {}
            xts = {}
            utns = {}
            e_nulls = {}

            def xcdma_gen(c):
                """raw x^T chunk load straight into projection layout,
                sliced per contraction block so projections can start as
                soon as the first block lands."""
                xc = xnt.tile([128, 8, 512], BF16, tag="xc", name="xc")
                xcs[c] = xc
                for k in range(8):
                    nc.sync.dma_start(
                        out=xc[:, k, :],
                        in_=xT_h[k * 128:(k + 1) * 128, c * 512:(c + 1) * 512],
                    )
                    if k % 4 == 3:
                        yield

            def xdma_gen(c, dq=None):
                """x-tile loads for chunk c's norm pass, issued a block
                ahead of the compute that consumes them."""
                ts = []
                for tr in range(4):
                    t = 4 * c + tr
                    xt = xin.tile([128, D], BF16, tag="xt", name="xt")
                    (dq or nc.sync).dma_start(
                        out=xt, in_=x_h[t * 128:(t + 1) * 128, :])
                    ts.append(xt)
                xts[c] = ts
                if False:
                    yield

            def norm_gen(c):
                """RMSNorm reciprocal factors: Square+accum on ACT from the
                row-major x copy, Newton-rsqrt on Pool; then fold r into the
                per-chunk rotary tables (cos_eff/sin_eff) and V row scales
                (rq)."""
                s0, s1 = c * 512, (c + 1) * 512
                for tr in range(4):
                    t = 4 * c + tr
                    xt = xts[c][tr]
                    ms = stat.tile([128, 1], F32, tag="ms", name="ms")
                    nc.scalar.activation(out=sqd, in_=xt, func=AF.Square,
                                         accum_out=ms)
                    mh = stat.tile([128, 1], F32, tag="mh", name="mh")
                    nc.gpsimd.tensor_scalar(
                        out=mh, in0=ms, scalar1=0.5 / D, scalar2=0.5 * EPS * EPS,
                        op0=OP.mult, op1=OP.max,
                    )
                    r = stat.tile([128, 1], F32, tag="r", name="r")
                    nc.gpsimd.tensor_scalar(
                        out=r, in0=mh, scalar1=0.0, scalar2=1.0,
                        op0=OP.mult, op1=OP.add,
                    )
                    for _ in range(2):
                        a = stat.tile([128, 1], F32, tag="a", name="a")
                        nc.gpsimd.tensor_mul(out=a, in0=r, in1=r)
                        nc.gpsimd.tensor_mul(out=a, in0=a, in1=mh)
                        nc.gpsimd.tensor_scalar(
                            out=a, in0=a, scalar1=-1.0, scalar2=1.5,
                            op0=OP.mult, op1=OP.add,
                        )
                        nc.gpsimd.tensor_mul(out=r, in0=r, in1=a)
                    nc.gpsimd.tensor_scalar_min(
                        out=rq[:, t:t + 1], in0=r, scalar1=1.0 / EPS)
                    yield
                # transpose the 4 per-partition columns into one query row via
                # a DRAM scratch (SBUF APs can't flip the partition axis)
                nc.sync.dma_start(
                    out=rscr_h[s0:s1].rearrange("(t p) -> p t", p=128),
                    in_=rq[:, 4 * c:4 * c + 4],
                )
                nc.sync.dma_start(out=rrow[0:1, s0:s1],
                                  in_=rscr_h[s0:s1].unsqueeze(0))
                rb = npool.tile([128, 512], F32, tag="rbq", name="rbq")
                nc.gpsimd.partition_broadcast(rb, rrow[0:1, s0:s1])
                yield
                nc.vector.tensor_mul(
                    out=cos_eff[:, s0:s1], in0=cos_sb[:, s0:s1], in1=rb)
                nc.vector.tensor_mul(
                    out=sin_eff[:, s0:s1], in0=sin_sb[:, s0:s1], in1=rb)
                yield

            def projrope_gen(c, mc):
                """q/k projection + RoPE for one head pair. Yields mid-group."""
                s0, s1 = c * 512, (c + 1) * 512
                xc = xcs[c]
                m0, m1 = mc * 128, (mc + 1) * 128
                for wsb, dst in ((wq_sb, qt), (wk_sb, kt)):
                    ps = pp.tile([128, 512], F32, tag="pp", name="ps")
                    for k in range(8):
                        nc.tensor.matmul(
                            ps, wsb[:, k, m0:m1], xc[:, k, :],
                            start=(k == 0), stop=(k == 7),
                        )
                        if k % 4 == 3:
                            yield
                    qraw = rope.tile([128, 512], BF16, tag="qraw", name="qraw")
                    nc.scalar.copy(out=qraw, in_=ps)
                    shuf = rope.tile([128, 512], BF16, tag="shuf", name="shuf")
                    nc.vector.stream_shuffle(
                        out=shuf, in_=qraw, mask=[i ^ 1 for i in range(32)]
                    )
                    qc = rope.tile([128, 512], BF16, tag="qc", name="qc")
                    nc.vector.tensor_mul(out=qc, in0=qraw, in1=cos_eff[:, s0:s1])
                    nc.gpsimd.tensor_tensor(
                        out=shuf, in0=shuf, in1=sin_eff[:, s0:s1], op=OP.mult,
                    )
                    nc.vector.tensor_add(
                        out=dst[mc][:, s0:s1], in0=qc, in1=shuf
                    )

            def vproj_gen(c):
                """V projections; key tile j holds V rows j*128..j*128+127."""
                xc = xcs[c]
                for tr in range(4):
                    j = 4 * c + tr
                    ps = pp.tile([128, CPH], F32, tag="pp", name="psv")
                    for k in range(8):
                        nc.tensor.matmul(
                            ps,
                            xc[:, k, tr * 128:(tr + 1) * 128],
                            wv_sb[:, k, :],
                            start=(k == 0), stop=(k == 7),
                        )
                        if k % 4 == 3:
                            yield
                    nc.scalar.mul(
                        out=vsb[j][:, :, 0:DH],
                        in_=ps.rearrange("p (h d) -> p h d", h=HPC),
                        mul=rq[:, j:j + 1],
                    )

            def advance(g, n=1):
                for _ in range(n):
                    try:
                        next(g)
                    except StopIteration:
                        return

            def drain(g):
                for _ in g:
                    pass

            def attn_mc(c, mc, filler, adv=2):
                """Attention for chunk c, head pair mc; braids `filler`
                pieces between key tiles."""
                s0, s1 = c * 512, (c + 1) * 512
                jl = 4 * c + 4
                uts = [
                    psu.tile([65, 512], F32, tag="ut", name=f"ut{hp}")
                    for hp in range(2)
                ]
                # null-kv rank-1 term starts both PSUM accumulations
                psn = pss.tile([33, 512], F32, tag="sp", name="psn")
                # hi=0 writes all 33 rows (cols 1..32 of its stationary are
                # zero) so the exp below never reads uninitialized PSUM;
                # hi=1 then overwrites row 32 with its real scores
                nc.tensor.matmul(
                    psn,
                    nk_sb[0:64, mc, :],
                    qt[mc][0:64, s0:s1],
                    start=True, stop=True,
                )
                nc.tensor.matmul(
                    psn[32:33, :],
                    nk_sb[64:128, mc, 0:1],
                    qt[mc][64:128, s0:s1],
                    start=True, stop=True,
                )
                e_null = enp.tile([33, 512], BF16, tag="en", name="en")
                nc.scalar.activation(out=e_null, in_=psn, func=AF.Exp)
                advance(filler, 1)
                pends = []
                nulldone = False
                for j in range(jl):
                    off = j - 4 * c
                    qlo = QLO[off] if off >= 0 else 0
                    sps = []
                    for hi in range(2):
                        hp = hi * 64
                        sp = pss.tile([128, 512], F32, tag="sp", name="sp")
                        nc.tensor.matmul(
                            sp[:, qlo:],
                            kt[mc][hp:hp + 64, j * 128:(j + 1) * 128],
                            qt[mc][hp:hp + 64, s0 + qlo:s1],
                            start=True, stop=True,
                        )
                        sps.append(sp)
                    if j == 2 or (jl <= 2 and j == jl - 1):
                        # null-kv rank-1 term opens both PSUM accumulations;
                        # emitted late so a stalled psu bank doesn't
                        # head-of-line-block the score matmuls
                        for hi in range(2):
                            nc.tensor.matmul(
                                uts[hi],
                                nv_sb[32 * hi:32 * hi + 1, mc, :],
                                e_null[32 * hi:32 * hi + 1, :],
                                start=True, stop=False,
                            )
                        nulldone = True
                    if len(pends) >= 2 and nulldone:
                        pj, pq, pe = pends.pop(0)
                        for hi in range(2):
                            nc.tensor.matmul(
                                uts[hi][:, pq:],
                                vsb[pj][:, 2 * mc + hi, :],
                                pe[:, hi, pq:],
                                start=False, stop=False,
                            )
                    e = epool.tile([128, 2, 512], BF16, tag="e", name="e")
                    nc.scalar.activation(
                        out=e[:, 0, qlo:], in_=sps[0][:, qlo:],
                        func=AF.Exp,
                        bias=mb_sb[:, j:j + 1], scale=1.0,
                    )
                    nc.vector.tensor_scalar(
                        out=e.bitcast(U16)[:, 1, qlo:],
                        in0=sps[1][:, qlo:],
                        scalar1=AFE, scalar2=bmb_sb[:, j:j + 1],
                        op0=OP.mult, op1=OP.add,
                    )
                    if off >= 0:
                        nc.vector.tensor_mul(
                            out=e[:, :, qlo:], in0=e[:, :, qlo:],
                            in1=masks[off][:, :, qlo:],
                        )
                    pends.append((j, qlo, e))
                    advance(filler, adv)
                while pends:
                    pj, pq, pe = pends.pop(0)
                    for hi in range(2):
                        nc.tensor.matmul(
                            uts[hi][:, pq:],
                            vsb[pj][:, 2 * mc + hi, :],
                            pe[:, hi, pq:],
                            start=False, stop=(not pends),
                        )
                # evacuate PSUM fast (frees the psu banks for the next pair);
                # the actual normalize is returned as a generator and braided
                # into the following attention block
                utn = upool.tile([128, 512], BF16, tag="utn", name="utn")
                urs = []
                for hi in range(2):
                    ur = npool.tile([65, 512], F32, tag="ur", name="ur")
                    if hi == 0:
                        nc.vector.tensor_copy(out=ur, in_=uts[hi])
                    else:
                        nc.scalar.copy(out=ur, in_=uts[hi])
                    urs.append(ur)
                utns[(c, mc)] = utn

                def normalize():
                    for hi in range(2):
                        r1_ = npool.tile([1, 512], F32, tag="r1", name="r1")
                        nc.vector.reciprocal(out=r1_, in_=urs[hi][64:65, :])
                        rb = npool.tile([64, 512], F32, tag="rb", name="rb")
                        nc.gpsimd.partition_broadcast(rb, r1_)
                        yield
                        nc.vector.tensor_mul(
                            out=utn[64 * hi:64 * hi + 64, :],
                            in0=urs[hi][0:64, :], in1=rb,
                        )
                        yield

                return normalize()

            def outproj_gen(c):
                s0, s1 = c * 512, (c + 1) * 512
                # chunk 3's out-proj runs after all attention: borrow the
                # then-idle triple-buffered score pool to avoid ypp stalls
                ypool = pss if c == NCI - 1 else ypp
                ytag = "sp" if c == NCI - 1 else "yp"
                for dc in range(8):
                    yp = ypool.tile([128, 512], F32, tag=ytag, name="yp")
                    for mc in range(2):
                        nc.tensor.matmul(
                            yp,
                            wo_sb[:, mc, dc * 128:(dc + 1) * 128],
                            utns[(c, mc)],
                            start=(mc == 0), stop=(mc == 1),
                        )
                        if mc == 0:
                            yield
                    ysb = yout.tile([128, 512], BF16, tag="ysb", name="ysb")
                    nc.scalar.copy(out=ysb[:, 0:256], in_=yp[:, 0:256])
                    nc.vector.tensor_copy(out=ysb[:, 256:], in_=yp[:, 256:])
                    nc.sync.dma_start(
                        out=yt_h[dc * 128:(dc + 1) * 128, s0:s1], in_=ysb
                    )
                    yield

            def chain(*gens):
                for g in gens:
                    yield from g

            def early_dmas():
                nc.sync.dma_start(
                    out=wq_sb, in_=wq_h.rearrange("(k p) c -> p k c", p=128))
                if False:
                    yield

            def cossin_dmas():
                nc.scalar.dma_start(out=cos_sb, in_=cos_h[:, :])
                nc.scalar.dma_start(out=sin_sb, in_=sin_h[:, :])
                if False:
                    yield

            def weights_dmas():
                nc.sync.dma_start(
                    out=wk_sb, in_=wk_h.rearrange("(k p) c -> p k c", p=128))
                nc.sync.dma_start(
                    out=wv_sb, in_=wv_h.rearrange("(k p) c -> p k c", p=128))
                nc.sync.dma_start(
                    out=nk_sb, in_=nk_h.rearrange("p (m w) -> p m w", m=2))
                nc.sync.dma_start(
                    out=nv_sb[0:1, :, 0:DH],
                    in_=nv_h[0:1, :].rearrange("o (m d) -> o m d", m=2))
                nc.sync.dma_start(
                    out=nv_sb[32:33, :, 0:DH],
                    in_=nv_h[1:2, :].rearrange("o (m d) -> o m d", m=2))
                nc.sync.dma_start(
                    out=mb_sb, in_=mb_h.rearrange("(t p) -> p t", p=128))
                nc.sync.dma_start(
                    out=bmb_sb, in_=bmb_h.rearrange("(t p) -> p t", p=128))
                nc.sync.dma_start(
                    out=wo_sb, in_=wo_h.rearrange("p (m c) -> p m c", m=2))
                if False:
                    yield

            # ---- driver: chunk-0 prep eager, then braided attention ----
            prep0 = chain(early_dmas(), xcdma_gen(0), cossin_dmas(),
                          xdma_gen(0, dq=nc.scalar), norm_gen(0),
                          weights_dmas(), projrope_gen(0, 0), vproj_gen(0))
            drain(prep0)
            pending_out = None
            for c in range(NCI):
                f1 = projrope_gen(c, 1)
                if pending_out is not None:
                    f1 = chain(f1, pending_out)
                fin0 = attn_mc(c, 0, f1, adv={0: 2, 1: 3, 2: 2, 3: 2}[c])
                drain(f1)
                if c < NCI - 1:
                    f2 = chain(fin0, xcdma_gen(c + 1), xdma_gen(c + 1),
                               norm_gen(c + 1), projrope_gen(c + 1, 0),
                               vproj_gen(c + 1))
                else:
                    f2 = fin0
                fin1 = attn_mc(c, 1, f2, adv={0: 5, 1: 4, 2: 3, 3: 1}[c])
                drain(f2)
                pending_out = chain(fin1, outproj_gen(c))
            drain(pending_out)

    nc.compile()
    return nc


def host_inputs(x, mask, freqs, g, Wq, Wkv, Wout, bout, null_kv):
    """Fold g/scale into weights and build the 8 per-core input dicts."""
    f32 = lambda a: np.ascontiguousarray(np.asarray(a, dtype=np.float32))
    bf16 = lambda a: np.ascontiguousarray(np.asarray(a).astype(np.float16))

    def round_f32r(a):
        """RNE-round fp32 to the PE's FP32R format (11-bit mantissa)."""
        b = np.ascontiguousarray(a, dtype=np.float32).view(np.uint32)
        b = (b + np.uint32(0x7FF) + ((b >> np.uint32(12)) & np.uint32(1))) & np.uint32(0xFFFFF000)
        return b.view(np.float32)
    x, freqs, g = f32(x), f32(freqs), f32(g)
    Wq, Wkv, Wout = f32(Wq), f32(Wkv), f32(Wout)
    null_kv = f32(null_kv)
    mask = np.asarray(mask, dtype=bool)

    scale = np.float32(DH ** -0.5)
    wq_eff = (Wq * g[:, None]) * scale
    wk_eff = Wkv[:, :H * DH] * g[:, None]
    wv_eff = Wkv[:, H * DH:] * g[:, None]

    cosT = np.ascontiguousarray(np.cos(freqs).T)
    sinT = np.sin(freqs).T.copy()
    sign = np.tile(np.array([-1.0, 1.0], np.float32), DH // 2)
    sinT *= sign[:, None]
    cos2 = np.ascontiguousarray(np.tile(cosT, (2, 1)))
    sin2 = np.ascontiguousarray(np.tile(sinT, (2, 1)))

    mbs, bmbs = [], []
    for b in range(B):
        mb = np.where(mask[b], 0.0, NEG).astype(np.float32)
        mbs.append(mb)
        bmbs.append((np.float32(BFE) + np.float32(AFE) * mb).astype(np.float32))

    nk_all = null_kv[0].reshape(H, DH)
    nv_all = null_kv[1].reshape(H, DH)

    in_maps = []
    for core in range(NCORES):
        b, hg = core // 4, core % 4
        h0 = hg * HPC
        # nk_dev[64*hi + d, mc] = nk[h0 + 2*mc + hi, d]
        nk_dev = np.zeros((128, 2, 33), np.float32)
        nv_dev = np.empty((2, 128), np.float32)
        wo_dev = np.empty((128, 2 * D), np.float32)
        for mc in range(2):
            for hi in range(2):
                h = h0 + 2 * mc + hi
                nk_dev[64 * hi:64 * hi + 64, mc, 0] = nk_all[h]
                nv_dev[hi, 64 * mc:64 * mc + 64] = nv_all[h]
                wo_dev[64 * hi:64 * hi + 64, mc * D:(mc + 1) * D] = \
                    Wout[h * DH:(h + 1) * DH, :]
        in_maps.append({
            "x": bf16(x[b]),
            "xT": bf16(np.ascontiguousarray(x[b].T)),
            "wq": bf16(wq_eff[:, h0 * DH:(h0 + HPC) * DH]),
            "wk": bf16(wk_eff[:, h0 * DH:(h0 + HPC) * DH]),
            "wv": bf16(wv_eff[:, h0 * DH:(h0 + HPC) * DH]),
            "wo": bf16(wo_dev),
            "cos2": bf16(cos2),
            "sin2": bf16(sin2),
            "nk": bf16(nk_dev.reshape(128, 2 * 33)),
            "nv": bf16(nv_dev),
            "mb": mbs[b],
            "bmb": bmbs[b],
        })
    return in_maps


_CACHE = {}


def kernel(**inputs):
    if "nc" not in _CACHE:
        _CACHE["nc"] = build_program()
    nc = _CACHE["nc"]

    in_maps = host_inputs(**inputs)

    from concourse.bass_utils import run_bass_kernel_spmd

    res = run_bass_kernel_spmd(
        nc, in_maps, core_ids=list(range(NCORES)), trace=TRACE, **TRACE_KW
    )
    _CACHE["last_result"] = res

    bout = np.asarray(inputs["bout"], dtype=np.float32)
    out = np.empty([B, N, D], np.float32)
    for b in range(B):
        acc = res.results[4 * b]["yt"].astype(np.float32)
        for c in range(4 * b + 1, 4 * b + 4):
            acc = acc + res.results[c]["yt"]
        out[b] = acc.T + bout
    return out


# revision 6
# speedup vs baseline: 1.3720x; 1.3720x over previous
"""Trainium2 Bass kernel for nn_Attention_21122649161959 (v6).

RETRO-style causal self-attention block (RMSNorm -> q/kv proj -> RoPE ->
null-kv prepend -> causal masked softmax -> out proj) for
x [2, 2048, 1024], 16 heads of 64.

Sharding: 8 NeuronCores = (batch 2) x (4 head-groups of 4 heads).
Each core computes, for its batch b and heads [h0, h0+4):
    y_partial^T = Wout[h-slice]^T @ attn_out^T          [1024, 2048]
The host sums the 4 (fp16) partials per batch, transposes, adds bias.

Design notes (v6; TimelineSim ~168 us/core vs 202 us for the v4
baseline; the wall is the PE sequencer, ~670 matmul issues x ~210 ns):
  - all-fp16 datapath (x, weights, q/k/v, rotary tables, outputs).
    fp16's 10-bit mantissa keeps the datapath error ~4e-4; bf16 was 8x
    worse at identical speed.  Weights cannot be f32r: the PE rejects
    mixed 32/16-bit operand pairs.
  - x is passed twice from the host: row-major for the RMSNorm pass
    (ACT Square+accum -> per-partition Newton rsqrt on Pool) and
    pre-transposed (xT) so projections read it directly -- no on-chip
    PE transposes.  The norm reciprocals fold into per-chunk scaled
    rotary tables (cos_eff/sin_eff) for q/k and into the V-tile copy
    (ACT activation scale) for V, so normalization costs no extra
    full-size elementwise pass.
  - de-shifted V tiles: key tile j covers real keys j*128..j*128+127;
    the causal diagonal lands exactly on tile edges (no sliver pass, no
    V boundary recompute).  The null kv is a rank-1 PSUM-opening update
    from a hoistable [1,512] score row + exp.
  - out projection packs head pairs into the full 128-partition
    contraction; softmax denominators ride along as a ones-column in
    the V tiles.
  - softmax exp is split across engines: ACT table exp for hi=0 heads,
    Schraudolph fast-exp (u16 = s*1024/ln2 + (15360-59), bitcast fp16)
    on DVE for hi=1 heads.  The sawtooth (~1.8% rms, zero-bias C) is
    the dominant error term (total rel err ~9e-3 vs the 2e-2 gate).
    fp8/DoubleRow variants were tested and rejected: fp8 weight
    quantization is a systematic W perturbation that does not average
    out over keys (measured 1.9e-2).
  - braided schedule (v4 lineage): projection/norm/output generators
    yield between PE groups and are advanced between attention key
    tiles; AV matmuls trail their scores by 2 tiles so exp latency
    never blocks the in-order PE queue; PSUM evacuations (u tiles) are
    copied out fast and normalized lazily in the next block's filler.
  - DMA: wq first, then per-k-block xT slices, cos/sin, the rest; the
    output stores are fp16 to halve the tail; outproj(3) borrows the
    idle triple-buffered score PSUM pool.
"""

import sys

sys.path.insert(0, "/opt/trn_rl_repo")

from contextlib import ExitStack

import numpy as np
import ml_dtypes

import concourse.bass as bass
import concourse.tile as tile
from concourse import bacc, mybir

F32 = mybir.dt.float32
F32R = mybir.dt.float32r
FP8 = mybir.dt.float8e4
DR = mybir.MatmulPerfMode.DoubleRow
BF16 = mybir.dt.float16
U16 = mybir.dt.uint16
AF = mybir.ActivationFunctionType
OP = mybir.AluOpType

B, N, D = 2, 2048, 1024
H, DH = 16, 64
HPC = 4
CPH = HPC * DH
NCORES = 8
NJT = 16
NCI = 4
NEG = -1e9
EPS = 1e-8
QLO = (0, 128, 256, 256)

# Schraudolph fast-exp in bf16 bit space: u16 = s * AFE + (BFE + AFE*mb)
AFE = 128.0 / np.log(2.0)
CFE = 4.1  # sawtooth centering constant (tuned numerically)
BFE = 127.0 * 128.0 - CFE

TRACE = False
TRACE_KW = {}


def build_program():
    nc = bacc.Bacc(trn_type="TRN2", num_devices=NCORES)

    x_h = nc.dram_tensor("x", [N, D], BF16, kind="ExternalInput")
    xT_h = nc.dram_tensor("xT", [D, N], BF16, kind="ExternalInput")
    wq_h = nc.dram_tensor("wq", [D, CPH], BF16, kind="ExternalInput")
    wk_h = nc.dram_tensor("wk", [D, CPH], BF16, kind="ExternalInput")
    wv_h = nc.dram_tensor("wv", [D, CPH], BF16, kind="ExternalInput")
    wo_h = nc.dram_tensor("wo", [128, 2 * D], BF16, kind="ExternalInput")
    cos_h = nc.dram_tensor("cos2", [128, N], BF16, kind="ExternalInput")
    sin_h = nc.dram_tensor("sin2", [128, N], BF16, kind="ExternalInput")
    nk_h = nc.dram_tensor("nk", [128, 2 * 33], BF16, kind="ExternalInput")
    nv_h = nc.dram_tensor("nv", [2, 128], BF16, kind="ExternalInput")
    mb_h = nc.dram_tensor("mb", [N], F32, kind="ExternalInput")
    bmb_h = nc.dram_tensor("bmb", [N], F32, kind="ExternalInput")
    yt_h = nc.dram_tensor("yt", [D, N], BF16, kind="ExternalOutput")
    rscr_h = nc.dram_tensor("rscr", [N], F32, kind="Internal")

    with ExitStack() as ctx:
        tc = ctx.enter_context(tile.TileContext(nc))
        persist = ctx.enter_context(tc.tile_pool(name="persist", bufs=1))

        def single(shape, tag, dt=F32):
            return persist.tile(shape, dt, tag=tag, name=tag)

        qt = [single([128, N], f"qt{m}", dt=BF16) for m in range(2)]
        kt = [single([128, N], f"kt{m}", dt=BF16) for m in range(2)]
        vsb = [single([128, HPC, DH + 1], f"v{j}", dt=BF16) for j in range(NJT)]
        mb_sb = single([128, NJT], "mb")
        bmb_sb = single([128, NJT], "bmb")
        cos_sb = single([128, N], "cos", dt=BF16)
        sin_sb = single([128, N], "sin", dt=BF16)
        cos_eff = single([128, N], "cose", dt=BF16)
        sin_eff = single([128, N], "sine", dt=BF16)
        rq = single([128, NJT], "rq")
        rrow = single([1, N], "rrow")
        sqd = single([128, D], "sqd", dt=BF16)

        wq_sb = single([128, 8, CPH], "wqs", dt=BF16)
        wk_sb = single([128, 8, CPH], "wks", dt=BF16)
        wv_sb = single([128, 8, CPH], "wvs", dt=BF16)
        wo_sb = single([128, 2, D], "wos", dt=BF16)
        nk_sb = single([128, 2, 33], "nks", dt=BF16)
        nv_sb = single([33, 2, DH + 1], "nvs", dt=BF16)

        # dummy exp first: pulls the ACT exp-table load off the critical path
        # without waiting on Pool's mask construction
        dexp = single([1, 1], "dexp")
        nc.vector.memset(dexp, 0.0)
        nc.scalar.activation(out=sqd[0:1, 0:1], in_=dexp, func=AF.Exp)

        masks = []
        for off in range(4):
            mt = single([128, 2, 512], f"mask{off}", dt=BF16)
            nc.gpsimd.memset(mt, 1.0)
            for hi in range(2):
                nc.gpsimd.affine_select(
                    out=mt[:, hi, :], in_=mt[:, hi, :], pattern=[[1, 512]],
                    compare_op=OP.is_ge, fill=0.0,
                    base=-off * 128, channel_multiplier=-1,
                )
            masks.append(mt)

        nc.vector.memset(nv_sb[:, :, DH:DH + 1], 1.0)
        for j in range(NJT):
            nc.vector.memset(vsb[j][:, :, DH:DH + 1], 1.0)

        with tc.tile_pool(name="xin", bufs=6) as xin, \
             tc.tile_pool(name="xnt", bufs=2) as xnt, \
             tc.tile_pool(name="stat", bufs=2) as stat, \
             tc.tile_pool(name="rope", bufs=3) as rope, \
             tc.tile_pool(name="epool", bufs=6) as epool, \
             tc.tile_pool(name="enp", bufs=2) as enp, \
             tc.tile_pool(name="npool", bufs=4) as npool, \
             tc.tile_pool(name="upool", bufs=4) as upool, \
             tc.tile_pool(name="yout", bufs=4) as yout, \
             tc.tile_pool(name="pp", bufs=2, space="PSUM") as pp, \
             tc.tile_pool(name="pss", bufs=3, space="PSUM") as pss, \
             tc.tile_pool(name="psu", bufs=2, space="PSUM") as psu, \
             tc.tile_pool(name="ypp", bufs=1, space="PSUM") as ypp:

            xcs = {}
            xts = {}
            utns = {}
            e_nulls = {}

            def xcdma_gen(c):
                """raw x^T chunk load straight into projection layout,
                sliced per contraction block so projections can start as
                soon as the first block lands."""
                xc = xnt.tile([128, 8, 512], BF16, tag="xc", name="xc")
                xcs[c] = xc
                for k in range(8):
                    nc.sync.dma_start(
                        out=xc[:, k, :],
                        in_=xT_h[k * 128:(k + 1) * 128, c * 512:(c + 1) * 512],
                    )
                    if k % 4 == 3:
                        yield

            def xdma_gen(c, dq=None):
                """x-tile loads for chunk c's norm pass, issued a block
                ahead of the compute that consumes them."""
                ts = []
                for tr in range(4):
                    t = 4 * c + tr
                    xt = xin.tile([128, D], BF16, tag="xt", name="xt")
                    (dq or nc.sync).dma_start(
                        out=xt, in_=x_h[t * 128:(t + 1) * 128, :])
                    ts.append(xt)
                xts[c] = ts
                if False:
                    yield

            def norm_gen(c):
                """RMSNorm reciprocal factors: Square+accum on ACT from the
                row-major x copy, Newton-rsqrt on Pool; then fold r into the
                per-chunk rotary tables (cos_eff/sin_eff) and V row scales
                (rq)."""
                s0, s1 = c * 512, (c + 1) * 512
                for tr in range(4):
                    t = 4 * c + tr
                    xt = xts[c][tr]
                    ms = stat.tile([128, 1], F32, tag="ms", name="ms")
                    nc.scalar.activation(out=sqd, in_=xt, func=AF.Square,
                                         accum_out=ms)
                    mh = stat.tile([128, 1], F32, tag="mh", name="mh")
                    nc.gpsimd.tensor_scalar(
                        out=mh, in0=ms, scalar1=0.5 / D, scalar2=0.5 * EPS * EPS,
                        op0=OP.mult, op1=OP.max,
                    )
                    r = stat.tile([128, 1], F32, tag="r", name="r")
                    nc.gpsimd.tensor_scalar(
                        out=r, in0=mh, scalar1=0.0, scalar2=1.0,
                        op0=OP.mult, op1=OP.add,
                    )
                    for _ in range(2):
                        a = stat.tile([128, 1], F32, tag="a", name="a")
                        nc.gpsimd.tensor_mul(out=a, in0=r, in1=r)
                        nc.gpsimd.tensor_mul(out=a, in0=a, in1=mh)
                        nc.gpsimd.tensor_scalar(
                            out=a, in0=a, scalar1=-1.0, scalar2=1.5,
                            op0=OP.mult, op1=OP.add,
                        )
                        nc.gpsimd.tensor_mul(out=r, in0=r, in1=a)
                    nc.gpsimd.tensor_scalar_min(
                        out=rq[:, t:t + 1], in0=r, scalar1=1.0 / EPS)
                    yield
                # transpose the 4 per-partition columns into one query row via
                # a DRAM scratch (SBUF APs can't flip the partition axis)
                nc.sync.dma_start(
                    out=rscr_h[s0:s1].rearrange("(t p) -> p t", p=128),
                    in_=rq[:, 4 * c:4 * c + 4],
                )
                nc.sync.dma_start(out=rrow[0:1, s0:s1],
                                  in_=rscr_h[s0:s1].unsqueeze(0))
                rb = npool.tile([128, 512], F32, tag="rbq", name="rbq")
                nc.gpsimd.partition_broadcast(rb, rrow[0:1, s0:s1])
                yield
                nc.vector.tensor_mul(
                    out=cos_eff[:, s0:s1], in0=cos_sb[:, s0:s1], in1=rb)
                nc.vector.tensor_mul(
                    out=sin_eff[:, s0:s1], in0=sin_sb[:, s0:s1], in1=rb)
                yield

            def projrope_gen(c, mc):
                """q/k projection + RoPE for one head pair. Yields mid-group."""
                s0, s1 = c * 512, (c + 1) * 512
                xc = xcs[c]
                m0, m1 = mc * 128, (mc + 1) * 128
                for wsb, dst in ((wq_sb, qt), (wk_sb, kt)):
                    ps = pp.tile([128, 512], F32, tag="pp", name="ps")
                    for k in range(8):
                        nc.tensor.matmul(
                            ps, wsb[:, k, m0:m1], xc[:, k, :],
                            start=(k == 0), stop=(k == 7),
                        )
                        if k % 4 == 3:
                            yield
                    qraw = rope.tile([128, 512], BF16, tag="qraw", name="qraw")
                    nc.scalar.copy(out=qraw, in_=ps)
                    shuf = rope.tile([128, 512], BF16, tag="shuf", name="shuf")
                    nc.vector.stream_shuffle(
                        out=shuf, in_=qraw, mask=[i ^ 1 for i in range(32)]
                    )
                    qc = rope.tile([128, 512], BF16, tag="qc", name="qc")
                    nc.vector.tensor_mul(out=qc, in0=qraw, in1=cos_eff[:, s0:s1])
                    nc.gpsimd.tensor_tensor(
                        out=shuf, in0=shuf, in1=sin_eff[:, s0:s1], op=OP.mult,
                    )
                    nc.vector.tensor_add(
                        out=dst[mc][:, s0:s1], in0=qc, in1=shuf
                    )

            def vproj_gen(c):
                """V projections; key tile j holds V rows j*128..j*128+127."""
                xc = xcs[c]
                for tr in range(4):
                    j = 4 * c + tr
                    ps = pp.tile([128, CPH], F32, tag="pp", name="psv")
                    for k in range(8):
                        nc.tensor.matmul(
                            ps,
                            xc[:, k, tr * 128:(tr + 1) * 128],
                            wv_sb[:, k, :],
                            start=(k == 0), stop=(k == 7),
                        )
                        if k % 4 == 3:
                            yield
                    nc.scalar.mul(
                        out=vsb[j][:, :, 0:DH],
                        in_=ps.rearrange("p (h d) -> p h d", h=HPC),
                        mul=rq[:, j:j + 1],
                    )

            def advance(g, n=1):
                for _ in range(n):
                    try:
                        next(g)
                    except StopIteration:
                        return

            def drain(g):
                for _ in g:
                    pass

            def attn_mc(c, mc, filler, adv=2):
                """Attention for chunk c, head pair mc; braids `filler`
                pieces between key tiles."""
                s0, s1 = c * 512, (c + 1) * 512
                jl = 4 * c + 4
                uts = [
                    psu.tile([65, 512], F32, tag="ut", name=f"ut{hp}")
                    for hp in range(2)
                ]
                # null-kv rank-1 term starts both PSUM accumulations
                psn = pss.tile([33, 512], F32, tag="sp", name="psn")
                # hi=0 writes all 33 rows (cols 1..32 of its stationary are
                # zero) so the exp below never reads uninitialized PSUM;
                # hi=1 then overwrites row 32 with its real scores
                nc.tensor.matmul(
                    psn,
                    nk_sb[0:64, mc, :],
                    qt[mc][0:64, s0:s1],
                    start=True, stop=True,
                )
                nc.tensor.matmul(
                    psn[32:33, :],
                    nk_sb[64:128, mc, 0:1],
                    qt[mc][64:128, s0:s1],
                    start=True, stop=True,
                )
                e_null = enp.tile([33, 512], BF16, tag="en", name="en")
                nc.scalar.activation(out=e_null, in_=psn, func=AF.Exp)
                advance(filler, 1)
                pends = []
                nulldone = False
                for j in range(jl):
                    off = j - 4 * c
                    qlo = QLO[off] if off >= 0 else 0
                    sps = []
                    for hi in range(2):
                        hp = hi * 64
                        sp = pss.tile([128, 512], F32, tag="sp", name="sp")
                        nc.tensor.matmul(
                            sp[:, qlo:],
                            kt[mc][hp:hp + 64, j * 128:(j + 1) * 128],
                            qt[mc][hp:hp + 64, s0 + qlo:s1],
                            start=True, stop=True,
                        )
                        sps.append(sp)
                    if j == 2 or (jl <= 2 and j == jl - 1):
                        # null-kv rank-1 term opens both PSUM accumulations;
                        # emitted late so a stalled psu bank doesn't
                        # head-of-line-block the score matmuls
                        for hi in range(2):
                            nc.tensor.matmul(
                                uts[hi],
                                nv_sb[32 * hi:32 * hi + 1, mc, :],
                                e_null[32 * hi:32 * hi + 1, :],
                                start=True, stop=False,
                            )
                        nulldone = True
                    if len(pends) >= 2 and nulldone:
                        pj, pq, pe = pends.pop(0)
                        for hi in range(2):
                            nc.tensor.matmul(
                                uts[hi][:, pq:],
                                vsb[pj][:, 2 * mc + hi, :],
                                pe[:, hi, pq:],
                                start=False, stop=False,
                            )
                    e = epool.tile([128, 2, 512], BF16, tag="e", name="e")
                    nc.scalar.activation(
                        out=e[:, 0, qlo:], in_=sps[0][:, qlo:],
                        func=AF.Exp,
                        bias=mb_sb[:, j:j + 1], scale=1.0,
                    )
                    nc.vector.tensor_scalar(
                        out=e.bitcast(U16)[:, 1, qlo:],
                        in0=sps[1][:, qlo:],
                        scalar1=AFE, scalar2=bmb_sb[:, j:j + 1],
                        op0=OP.mult, op1=OP.add,
                    )
                    if off >= 0:
                        nc.vector.tensor_mul(
                            out=e[:, :, qlo:], in0=e[:, :, qlo:],
                            in1=masks[off][:, :, qlo:],
                        )
                    pends.append((j, qlo, e))
                    advance(filler, adv)
                while pends:
                    pj, pq, pe = pends.pop(0)
                    for hi in range(2):
                        nc.tensor.matmul(
                            uts[hi][:, pq:],
                            vsb[pj][:, 2 * mc + hi, :],
                            pe[:, hi, pq:],
                            start=False, stop=(not pends),
                        )
                # evacuate PSUM fast (frees the psu banks for the next pair);
                # the actual normalize is returned as a generator and braided
                # into the following attention block
                utn = upool.tile([128, 512], BF16, tag="utn", name="utn")
                urs = []
                for hi in range(2):
                    ur = npool.tile([65, 512], F32, tag="ur", name="ur")
                    if hi == 0:
                        nc.vector.tensor_copy(out=ur, in_=uts[hi])
                    else:
                        nc.scalar.copy(out=ur, in_=uts[hi])
                    urs.append(ur)
                utns[(c, mc)] = utn

                def normalize():
                    for hi in range(2):
                        r1_ = npool.tile([1, 512], F32, tag="r1", name="r1")
                        nc.vector.reciprocal(out=r1_, in_=urs[hi][64:65, :])
                        rb = npool.tile([64, 512], F32, tag="rb", name="rb")
                        nc.gpsimd.partition_broadcast(rb, r1_)
                        yield
                        nc.vector.tensor_mul(
                            out=utn[64 * hi:64 * hi + 64, :],
                            in0=urs[hi][0:64, :], in1=rb,
                        )
                        yield

                return normalize()

            def outproj_gen(c):
                s0, s1 = c * 512, (c + 1) * 512
                # chunk 3's out-proj runs after all attention: borrow the
                # then-idle triple-buffered score pool to avoid ypp stalls
                ypool = pss if c == NCI - 1 else ypp
                ytag = "sp" if c == NCI - 1 else "yp"
                for dc in range(8):
                    yp = ypool.tile([128, 512], F32, tag=ytag, name="yp")
                    for mc in range(2):
                        nc.tensor.matmul(
                            yp,
                            wo_sb[:, mc, dc * 128:(dc + 1) * 128],
                            utns[(c, mc)],
                            start=(mc == 0), stop=(mc == 1),
                        )
                        if mc == 0:
                            yield
                    ysb = yout.tile([128, 512], BF16, tag="ysb", name="ysb")
                    nc.scalar.copy(out=ysb[:, 0:256], in_=yp[:, 0:256])
                    nc.vector.tensor_copy(out=ysb[:, 256:], in_=yp[:, 256:])
                    nc.sync.dma_start(
                        out=yt_h[dc * 128:(dc + 1) * 128, s0:s1], in_=ysb
                    )
                    yield

            def chain(*gens):
                for g in gens:
                    yield from g

            def early_dmas():
                nc.sync.dma_start(
                    out=wq_sb, in_=wq_h.rearrange("(k p) c -> p k c", p=128))
                if False:
                    yield

            def cossin_dmas():
                nc.scalar.dma_start(out=cos_sb, in_=cos_h[:, :])
                nc.scalar.dma_start(out=sin_sb, in_=sin_h[:, :])
                if False:
                    yield

            def weights_dmas():
                nc.sync.dma_start(
                    out=wk_sb, in_=wk_h.rearrange("(k p) c -> p k c", p=128))
                nc.sync.dma_start(
                    out=wv_sb, in_=wv_h.rearrange("(k p) c -> p k c", p=128))
                nc.sync.dma_start(
                    out=nk_sb, in_=nk_h.rearrange("p (m w) -> p m w", m=2))
                nc.sync.dma_start(
                    out=nv_sb[0:1, :, 0:DH],
                    in_=nv_h[0:1, :].rearrange("o (m d) -> o m d", m=2))
                nc.sync.dma_start(
                    out=nv_sb[32:33, :, 0:DH],
                    in_=nv_h[1:2, :].rearrange("o (m d) -> o m d", m=2))
                nc.sync.dma_start(
                    out=mb_sb, in_=mb_h.rearrange("(t p) -> p t", p=128))
                nc.sync.dma_start(
                    out=bmb_sb, in_=bmb_h.rearrange("(t p) -> p t", p=128))
                nc.sync.dma_start(
                    out=wo_sb, in_=wo_h.rearrange("p (m c) -> p m c", m=2))
                if False:
                    yield

            # ---- driver: chunk-0 prep eager, then braided attention ----
            prep0 = chain(early_dmas(), xcdma_gen(0), cossin_dmas(),
                          xdma_gen(0, dq=nc.scalar), norm_gen(0),
                          weights_dmas(), projrope_gen(0, 0), vproj_gen(0))
            drain(prep0)
            pending_out = None
            for c in range(NCI):
                f1 = projrope_gen(c, 1)
                if pending_out is not None:
                    f1 = chain(f1, pending_out)
                fin0 = attn_mc(c, 0, f1, adv={0: 2, 1: 3, 2: 2, 3: 2}[c])
                drain(f1)
                if c < NCI - 1:
                    f2 = chain(fin0, xcdma_gen(c + 1), xdma_gen(c + 1),
                               norm_gen(c + 1), projrope_gen(c + 1, 0),
                               vproj_gen(c + 1))
                else:
                    f2 = fin0
                fin1 = attn_mc(c, 1, f2, adv={0: 5, 1: 4, 2: 3, 3: 1}[c])
                drain(f2)
                pending_out = chain(fin1, outproj_gen(c))
            drain(pending_out)

    nc.compile()
    return nc


def host_inputs(x, mask, freqs, g, Wq, Wkv, Wout, bout, null_kv):
    """Fold g/scale into weights and build the 8 per-core input dicts."""
    f32 = lambda a: np.ascontiguousarray(np.asarray(a, dtype=np.float32))
    bf16 = lambda a: np.ascontiguousarray(np.asarray(a).astype(np.float16))

    def round_f32r(a):
        """RNE-round fp32 to the PE's FP32R format (11-bit mantissa)."""
        b = np.ascontiguousarray(a, dtype=np.float32).view(np.uint32)
        b = (b + np.uint32(0x7FF) + ((b >> np.uint32(12)) & np.uint32(1))) & np.uint32(0xFFFFF000)
        return b.view(np.float32)
    x, freqs, g = f32(x), f32(freqs), f32(g)
    Wq, Wkv, Wout = f32(Wq), f32(Wkv), f32(Wout)
    null_kv = f32(null_kv)
    mask = np.asarray(mask, dtype=bool)

    scale = np.float32(DH ** -0.5)
    wq_eff = (Wq * g[:, None]) * scale
    wk_eff = Wkv[:, :H * DH] * g[:, None]
    wv_eff = Wkv[:, H * DH:] * g[:, None]

    cosT = np.ascontiguousarray(np.cos(freqs).T)
    sinT = np.sin(freqs).T.copy()
    sign = np.tile(np.array([-1.0, 1.0], np.float32), DH // 2)
    sinT *= sign[:, None]
    cos2 = np.ascontiguousarray(np.tile(cosT, (2, 1)))
    sin2 = np.ascontiguousarray(np.tile(sinT, (2, 1)))

    mbs, bmbs = [], []
    for b in range(B):
        mb = np.where(mask[b], 0.0, NEG).astype(np.float32)
        mbs.append(mb)
        bmbs.append((np.float32(BFE) + np.float32(AFE) * mb).astype(np.float32))

    nk_all = null_kv[0].reshape(H, DH)
    nv_all = null_kv[1].reshape(H, DH)

    in_maps = []
    for core in range(NCORES):
        b, hg = core // 4, core % 4
        h0 = hg * HPC
        # nk_dev[64*hi + d, mc] = nk[h0 + 2*mc + hi, d]
        nk_dev = np.zeros((128, 2, 33), np.float32)
        nv_dev = np.empty((2, 128), np.float32)
        wo_dev = np.empty((128, 2 * D), np.float32)
        for mc in range(2):
            for hi in range(2):
                h = h0 + 2 * mc + hi
                nk_dev[64 * hi:64 * hi + 64, mc, 0] = nk_all[h]
                nv_dev[hi, 64 * mc:64 * mc + 64] = nv_all[h]
                wo_dev[64 * hi:64 * hi + 64, mc * D:(mc + 1) * D] = \
                    Wout[h * DH:(h + 1) * DH, :]
        in_maps.append({
            "x": bf16(x[b]),
            "xT": bf16(np.ascontiguousarray(x[b].T)),
            "wq": bf16(wq_eff[:, h0 * DH:(h0 + HPC) * DH]),
            "wk": bf16(wk_eff[:, h0 * DH:(h0 + HPC) * DH]),
            "wv": bf16(wv_eff[:, h0 * DH:(h0 + HPC) * DH]),
            "wo": bf16(wo_dev),
            "cos2": bf16(cos2),
            "sin2": bf16(sin2),
            "nk": bf16(nk_dev.reshape(128, 2 * 33)),
            "nv": bf16(nv_dev),
            "mb": mbs[b],
            "bmb": bmbs[b],
        })
    return in_maps


_CACHE = {}


def kernel(**inputs):
    if "nc" not in _CACHE:
        _CACHE["nc"] = build_program()
    nc = _CACHE["nc"]

    in_maps = host_inputs(**inputs)

    from concourse.bass_utils import run_bass_kernel_spmd

    res = run_bass_kernel_spmd(
        nc, in_maps, core_ids=list(range(NCORES)), trace=TRACE, **TRACE_KW
    )
    _CACHE["last_result"] = res

    bout = np.asarray(inputs["bout"], dtype=np.float32)
    out = np.empty([B, N, D], np.float32)
    for b in range(B):
        acc = res.results[4 * b]["yt"].astype(np.float32)
        for c in range(4 * b + 1, 4 * b + 4):
            acc = acc + res.results[c]["yt"]
        out[b] = acc.T + bout
    return out
